# revision 16
# baseline (speedup 1.0000x reference)
"""GroupedQueryAttention kernel for 8 Trainium2 NeuronCores.

Sharding: core c = (batch b = c//2, seq-half sh = c%2). Each core computes the
full attention output for 1024 query rows of one batch: all 8 q heads
(2 kv heads), plus the q/k/v projections and the o-projection for those rows.
Host work is limited to slicing/transposing/casting inputs and concatenating
outputs.

On-device layout: scoresT [keys, queries] so softmax-exp'd probabilities feed
attn@v matmuls directly as the moving operand.

The kernel is softmax-exp bound: the Scalar engine must evaluate
H*SQ*S = 16.8M exps per core (~1.1us per [128,1024] block, 128 blocks).
Everything else is scheduled around keeping that pipeline saturated:

- The matmul path runs in bf16 (1 PE cycle/row vs ~4 for fp32); PSUM
  accumulation stays fp32.
- Softmax denominators ride along in the attn@v matmuls: the V stationary
  carries a 65th column of ones, so row 64 of each accumulator is the
  denominator for free.
- Denominator rows are DMA'd from PSUM partition 64 to SBUF partition 0
  (engines cannot shift partitions; DMA can), reciprocal'd there, then
  partition-broadcast on the otherwise-idle GPSIMD engine (whose ucode
  requires a partition-0 source); normalization is then a plain multiply.
- The attn output halves are assembled into one [128,512] tile via an
  SBUF->SBUF DMA partition shift so o-proj contracts over all 128 dims.
- x/k/v/q tensors are tiled per chunk so dependencies are fine-grained, and
  all projection + o-proj work that is not needed immediately is drained
  1-2 items per kb slot into the PE idle gaps of the exp-bound attention
  loops ("deferred work"), instead of running as serial phases.
"""

import numpy as np

B, S, D = 4, 2048, 512
H, KV, DH = 8, 2, 64
SQ = S // 2  # queries per core
NCORES = 8
PAIRS = 4  # head pairs (p, p+4); p -> kv0 rows 0:64, p+4 -> kv1 rows 64:128
SCALE = 1.0 / 8.0  # 1/sqrt(DH)
PERM = [0, 4, 1, 5, 2, 6, 3, 7]  # q head order: pair-major
NKB = S // 128  # 16 key blocks
NSC = S // 512  # 4 column chunks of x

_built = {}


def _build_nc():
    import concourse.mybir as mybir
    import concourse.tile as tile
    from concourse import bacc

    fp32 = mybir.dt.float32
    bf16 = mybir.dt.bfloat16
    Exp = mybir.ActivationFunctionType.Exp

    nc = bacc.Bacc("TRN2", target_bir_lowering=False, debug=False,
                   num_devices=NCORES)

    # all matrices arrive pre-arranged on the host into the exact SBUF
    # layout [partition, chunk, col] so every input DMA is fully contiguous
    xp = nc.dram_tensor("xp", [128, NSC * 4 * 512], bf16,
                        kind="ExternalInput").ap()
    wq = nc.dram_tensor("wq", [128, 4 * D], bf16, kind="ExternalInput").ap()
    wk = nc.dram_tensor("wk", [128, 4 * 128], bf16, kind="ExternalInput").ap()
    wv = nc.dram_tensor("wv", [128, 4 * 128], bf16, kind="ExternalInput").ap()
    wo = nc.dram_tensor("wo", [128, 4 * D], bf16, kind="ExternalInput").ap()
    bqp = nc.dram_tensor("bqp", [128, PAIRS], fp32, kind="ExternalInput").ap()
    bkvp = nc.dram_tensor("bkvp", [128, 1], fp32, kind="ExternalInput").ap()
    bvbc = nc.dram_tensor("bvbc", [128, 128], fp32, kind="ExternalInput").ap()
    bobc = nc.dram_tensor("bobc", [128, D], fp32, kind="ExternalInput").ap()
    y = nc.dram_tensor("y", [SQ, D], fp32, kind="ExternalOutput").ap()

    with tile.TileContext(nc) as tc:
        with (
            tc.tile_pool(name="consts", bufs=1) as consts,
            tc.tile_pool(name="epool", bufs=3) as epool,
            tc.tile_pool(name="opool", bufs=9) as opool,
            tc.tile_pool(name="obpool", bufs=3) as obpool,
            tc.tile_pool(name="npool", bufs=4) as npool,
            tc.tile_pool(name="bcpool", bufs=4) as bcpool,
            tc.tile_pool(name="ypool", bufs=3) as ypool,
            tc.tile_pool(name="pssc", bufs=2, space="PSUM") as pssc,
            tc.tile_pool(name="pacc", bufs=4, space="PSUM") as pacc,
        ):
            # ---- input DMAs: split across the two HWDGE queues (SP and
            # Activation -- the scalar engine is idle during the prologue)
            # so transfers overlap; everything contiguous via host prearrange
            wk_sb = consts.tile([128, 4, 128], bf16, tag="wk")
            nc.sync.dma_start(wk_sb[:], wk.rearrange("p (c j) -> p c j", c=4))
            xt_ch = []
            for sc in range(NSC):
                xch = consts.tile([128, 4, 512], bf16, name=f"xch{sc}",
                                  tag=f"xt{sc}")
                xt_ch.append(xch)
            nc.sync.dma_start(xt_ch[0][:],
                              xp[:, 0:2048].rearrange("p (c j) -> p c j", c=4))
            wq_sb = consts.tile([128, 4, D], bf16, tag="wq")
            nc.sync.dma_start(wq_sb[:], wq.rearrange("p (c j) -> p c j", c=4))
            bq_sb = consts.tile([128, PAIRS], fp32, tag="bq")
            nc.sync.dma_start(bq_sb[:], bqp)
            bkv_sb = consts.tile([128, 1], fp32, tag="bkv")
            nc.sync.dma_start(bkv_sb[:], bkvp)
            nc.sync.dma_start(xt_ch[1][:],
                              xp[:, 2048:4096].rearrange("p (c j) -> p c j", c=4))
            wv_sb = consts.tile([128, 4, 128], bf16, tag="wv")
            nc.scalar.dma_start(wv_sb[:], wv.rearrange("p (c j) -> p c j", c=4))
            bv_sb = consts.tile([128, 128], fp32, tag="bv")
            nc.scalar.dma_start(bv_sb[:], bvbc)
            nc.scalar.dma_start(xt_ch[2][:],
                                xp[:, 4096:6144].rearrange("p (c j) -> p c j", c=4))
            nc.scalar.dma_start(xt_ch[3][:],
                                xp[:, 6144:8192].rearrange("p (c j) -> p c j", c=4))
            wo_sb = consts.tile([128, 4, D], bf16, tag="wo")
            nc.scalar.dma_start(wo_sb[:], wo.rearrange("p (c j) -> p c j", c=4))
            bo_sb = consts.tile([128, D], fp32, tag="bo")
            nc.scalar.dma_start(bo_sb[:], bobc)

            # per-chunk kT tiles, per-block V tiles, per-pair qT tiles so
            # consumers wait only on the piece they need
            ktt = [consts.tile([128, 512], bf16, name=f"ktt{sc}",
                               tag=f"kt{sc}") for sc in range(NSC)]
            # V block: cols 0:64 = v_kv0, 64 = ones, 65:129 = v_kv1, 129 = ones
            vpt = [consts.tile([128, 130], bf16, name=f"vpt{sb}",
                               tag=f"vp{sb}") for sb in range(NKB)]
            qtt = [consts.tile([128, SQ], bf16, name=f"qtt{pr}",
                               tag=f"qt{pr}") for pr in range(PAIRS)]

            # Projection / o-proj emitters. `ps` is the PSUM region to use:
            # in the prologue a pacc tile, inside attention jobs a 512-col
            # half of the PREVIOUS slot's scores tile (already read by its
            # exp, and the next writer is 2 slots away in PE program order).
            def kt_proj(sc, ps):
                for c in range(4):
                    nc.tensor.matmul(ps[:, 0:512], wk_sb[:, c, :],
                                     xt_ch[sc][:, c, :],
                                     start=(c == 0), stop=(c == 3))
                nc.vector.tensor_scalar_add(ktt[sc][:], ps[:, 0:512],
                                            bkv_sb[:, 0:1])

            def v_proj(sb, ps):
                xch = xt_ch[sb // 4]
                off = (sb % 4) * 128
                for c in range(4):
                    nc.tensor.matmul(ps[:, 0:128],
                                     xch[:, c, off:off + 128],
                                     wv_sb[:, c, :],
                                     start=(c == 0), stop=(c == 3))
                nc.vector.memset(vpt[sb][:, 64:65], 1.0)
                nc.vector.memset(vpt[sb][:, 129:130], 1.0)
                nc.vector.tensor_add(vpt[sb][:, 0:64], ps[:, 0:64],
                                     bv_sb[:, 0:64])
                nc.vector.tensor_add(vpt[sb][:, 65:129], ps[:, 64:128],
                                     bv_sb[:, 64:128])

            def qt_proj(pr, sc, ps):
                for c in range(4):
                    nc.tensor.matmul(ps[:, 0:512],
                                     wq_sb[:, c, pr * 128:(pr + 1) * 128],
                                     xt_ch[sc][:, c, :],
                                     start=(c == 0), stop=(c == 3))
                nc.vector.tensor_scalar_add(
                    qtt[pr][:, sc * 512:(sc + 1) * 512], ps[:, 0:512],
                    bq_sb[:, pr:pr + 1])

            ot_tiles = {}  # (qc, pr) -> assembled [128, 512] bf16 attn out

            def oproj_m(qc, m, ps):
                for pr in range(PAIRS):
                    nc.tensor.matmul(ps[:, 0:512],
                                     ot_tiles[(qc, pr)][:, m * 128:(m + 1) * 128],
                                     wo_sb[:, pr, :],
                                     start=(pr == 0), stop=(pr == 3))
                yt = ypool.tile([128, 512], fp32, tag="y")
                nc.vector.tensor_add(yt[:], ps[:, 0:512], bo_sb[:])
                blk = qc * 4 + m
                nc.sync.dma_start(y[blk * 128:(blk + 1) * 128, :], yt[:])

            def with_pacc(fn, *args):
                ps = pacc.tile([128, 512], fp32, tag="pacc")
                fn(*args, ps[:])

            # ---- serial prologue: minimum work before the exp pipeline can
            # start (kT chunk 0, V blocks 0-3, kT chunk 1, qT pair 0) ----
            with_pacc(kt_proj, 0)
            with_pacc(qt_proj, 0, 0)
            with_pacc(qt_proj, 0, 1)
            for sb in range(4):
                with_pacc(v_proj, sb)
            with_pacc(kt_proj, 1)

            # deferred work drained into the attention loops' PE idle slots:
            # {job: {slot: [closure(ps), ...]}}.  Slots 0-1 have no prev
            # scores tile in job 0, so job-0 items start at slot 2.
            def item(fn, *args):
                return lambda ps: fn(*args, ps)

            deferred = {j: {} for j in range(8)}

            def defer(j, slot, fn, *args):
                deferred[j].setdefault(slot, []).append(item(fn, *args))

            defer(0, 4, kt_proj, 2)
            defer(0, 5, kt_proj, 3)
            for sb in range(4, NKB):  # vp(k) needed by attnv(k) at slot k+1
                defer(0, sb - 2, v_proj, sb)
            defer(1, 4, qt_proj, 1, 0)
            defer(1, 6, qt_proj, 1, 1)
            defer(3, 4, qt_proj, 2, 0)
            defer(3, 6, qt_proj, 2, 1)
            defer(5, 4, qt_proj, 3, 0)
            defer(5, 6, qt_proj, 3, 1)
            # o-proj for qc0 hides in the last job (after ot(0,p3) is ready)
            defer(7, 6, oproj_m, 0, 0)
            defer(7, 9, oproj_m, 0, 1)
            defer(7, 12, oproj_m, 0, 2)
            defer(7, 15, oproj_m, 0, 3)

            # ---- 8 attention jobs: qc-major within pair so each pair's qT
            # is reused by consecutive jobs ----
            jobs = [(qc, pr) for pr in range(PAIRS) for qc in range(2)]
            prev_sc = [None, None]
            for j, (qc, pr) in enumerate(jobs):
                pA = pacc.tile([128, 512], fp32, tag="pacc")
                pB = pacc.tile([128, 512], fp32, tag="pacc")
                e_tiles = [None] * NKB

                def attnv(kb):
                    e = e_tiles[kb]
                    nc.tensor.matmul(pA[0:65, :], vpt[kb][:, 0:65],
                                     e[:, 0:512],
                                     start=(kb == 0), stop=(kb == NKB - 1))
                    nc.tensor.matmul(pB[0:65, :], vpt[kb][:, 65:130],
                                     e[:, 512:1024],
                                     start=(kb == 0), stop=(kb == NKB - 1))

                for kb in range(NKB):
                    sc_ps = pssc.tile([128, 1024], fp32, tag="scores")
                    nc.tensor.matmul(
                        sc_ps[:, 0:512],
                        ktt[kb // 4][0:64, (kb % 4) * 128:(kb % 4 + 1) * 128],
                        qtt[pr][0:64, qc * 512:(qc + 1) * 512])
                    nc.tensor.matmul(
                        sc_ps[:, 512:1024],
                        ktt[kb // 4][64:128, (kb % 4) * 128:(kb % 4 + 1) * 128],
                        qtt[pr][64:128, qc * 512:(qc + 1) * 512])
                    e = epool.tile([128, 1024], bf16, tag="E")
                    e_tiles[kb] = e
                    nc.scalar.activation(e[:], sc_ps[:], Exp, scale=SCALE)
                    # consume the previous block's probs so PE never waits on
                    # the exp of the current block
                    if kb >= 1:
                        attnv(kb - 1)
                    items = deferred[j].get(kb, ())
                    if items:
                        for i, fn in enumerate(items):
                            fn(prev_sc[0][:, i * 512:(i + 1) * 512])
                    prev_sc[0] = prev_sc[1]
                    prev_sc[1] = sc_ps
                attnv(NKB - 1)

                # normalize: den_p in pA row 64, den_p+4 in pB row 64.
                # Engines cannot shift partitions and the gpsimd broadcast
                # ucode reads its source from partition 0 only, so copy the
                # rows out of PSUM and DMA them down to partition 0.
                dsb = npool.tile([65, 1024], fp32, tag="den64")
                nc.vector.tensor_copy(dsb[64:65, 0:512], pA[64:65, :])
                nc.vector.tensor_copy(dsb[64:65, 512:1024], pB[64:65, :])
                d0 = npool.tile([1, 1024], fp32, tag="den0")
                nc.sync.dma_start(d0[0:1, :], dsb[64:65, :])
                r0 = npool.tile([1, 1024], fp32, tag="rden0")
                s0 = npool.tile([1, 1024], fp32, tag="rscr0")
                nc.vector.reciprocal_approx_accurate(r0[:], d0[:], s0[:])
                rbcA = bcpool.tile([64, 512], fp32, tag="rbcA")
                rbcB = bcpool.tile([64, 512], fp32, tag="rbcB")
                nc.gpsimd.partition_broadcast(rbcA[:], r0[0:1, 0:512],
                                              channels=64)
                nc.gpsimd.partition_broadcast(rbcB[:], r0[0:1, 512:1024],
                                              channels=64)
                # assemble both normalized halves into one [128, 512] tile
                # (otB via DMA partition shift) so o-proj contracts over 128
                ot = opool.tile([128, 512], bf16, tag="ot")
                nc.vector.tensor_mul(ot[0:64, :], pA[0:64, :], rbcA[:])
                obt = obpool.tile([64, 512], bf16, tag="obt")
                nc.vector.tensor_mul(obt[:], pB[0:64, :], rbcB[:])
                nc.sync.dma_start(ot[64:128, :], obt[:])
                ot_tiles[(qc, pr)] = ot

            # tail: o-proj for qc1 (qc0's was drained into job 7).  The
            # pr=0..2 partial accumulations only need ots that are long
            # ready, so they run while job 7's normalize chain produces
            # ot(1,p3); only the final pr=3 matmuls wait on it.
            yps = []
            for m in range(4):
                if m < 2:
                    ps = pacc.tile([128, 512], fp32, name=f"ypt{m}",
                                   tag="pacc")[:]
                else:
                    ps = pssc.tile([128, 1024], fp32, name=f"ypt{m}",
                                   tag="scores")[:, 0:512]
                yps.append(ps)
                for pr in range(3):
                    nc.tensor.matmul(ps[:, 0:512],
                                     ot_tiles[(1, pr)][:, m * 128:(m + 1) * 128],
                                     wo_sb[:, pr, :],
                                     start=(pr == 0), stop=False)
            for m in range(4):
                ps = yps[m]
                nc.tensor.matmul(ps[:, 0:512],
                                 ot_tiles[(1, 3)][:, m * 128:(m + 1) * 128],
                                 wo_sb[:, 3, :],
                                 start=False, stop=True)
                yt = ypool.tile([128, 512], fp32, name=f"ytt{m}", tag="y")
                nc.vector.tensor_add(yt[:], ps[:, 0:512], bo_sb[:])
                blk = 4 + m
                nc.sync.dma_start(y[blk * 128:(blk + 1) * 128, :], yt[:])

    nc.finalize()
    return nc


def _get_nc():
    if "nc" not in _built:
        _built["nc"] = _build_nc()
    return _built["nc"]


def _in_maps(x, Wq, bq, Wk, bk, Wv, bv, Wo, bo):
    import ml_dtypes

    b16 = ml_dtypes.bfloat16
    x = np.ascontiguousarray(np.asarray(x, np.float32))
    Wq = np.asarray(Wq, np.float32)
    bq = np.asarray(bq, np.float32)
    Wk = np.asarray(Wk, np.float32)
    bk = np.asarray(bk, np.float32)
    Wv = np.asarray(Wv, np.float32)
    bv = np.asarray(bv, np.float32)
    Wo = np.asarray(Wo, np.float32)
    bo = np.asarray(bo, np.float32)

    def chunked(a):  # [D, n] row-major -> [128, 4*n] with row d = (c, p)
        n = a.shape[1]
        return np.ascontiguousarray(
            a.reshape(4, 128, n).transpose(1, 0, 2).reshape(128, 4 * n))

    wq_p = chunked(
        Wq.reshape(D, H, DH)[:, PERM, :].reshape(D, D)).astype(b16)
    wo_p = chunked(
        Wo.reshape(H, DH, D)[PERM].reshape(D, D)).astype(b16)
    wk_p = chunked(Wk).astype(b16)
    wv_p = chunked(Wv).astype(b16)
    bq_p = np.ascontiguousarray(
        bq.reshape(H, DH)[PERM].reshape(PAIRS, 128).T)
    bkv_p = np.ascontiguousarray(bk.reshape(128, 1))
    bv_bc = np.ascontiguousarray(np.tile(bv[None, :], (128, 1)))
    bo_bc = np.ascontiguousarray(np.tile(bo[None, :], (128, 1)))

    in_maps = []
    for c in range(NCORES):
        b, sh = divmod(c, 2)
        xroll = np.roll(x[b], -sh * SQ, axis=0)
        # xT [D, S] -> [128, sc, c, 512] chunk-major contiguous
        xprep = np.ascontiguousarray(
            xroll.T.reshape(4, 128, 4, 512).transpose(1, 2, 0, 3)
            .reshape(128, NSC * 4 * 512)).astype(b16)
        in_maps.append({
            "xp": xprep,
            "wq": wq_p, "wk": wk_p, "wv": wv_p, "wo": wo_p,
            "bqp": bq_p, "bkvp": bkv_p, "bvbc": bv_bc, "bobc": bo_bc,
        })
    return in_maps


def kernel(x, Wq, bq, Wk, bk, Wv, bv, Wo, bo):
    from concourse.bass_utils import run_bass_kernel_spmd

    in_maps = _in_maps(x, Wq, bq, Wk, bk, Wv, bv, Wo, bo)
    nc = _get_nc()
    res = run_bass_kernel_spmd(nc, in_maps, list(range(NCORES)))
    out = np.empty((B, S, D), np.float32)
    for c in range(NCORES):
        b, sh = divmod(c, 2)
        out[b, sh * SQ:(sh + 1) * SQ, :] = res.results[c]["y"]
    return out


# revision 21
# speedup vs baseline: 1.1550x; 1.1550x over previous
"""GroupedQueryAttention kernel for 8 Trainium2 NeuronCores.

Sharding: core c = (batch b = c//2, seq-half sh = c%2). Each core computes the
full attention output for 1024 query rows of one batch: all 8 q heads
(2 kv heads), plus the q/k/v projections and the o-projection for those rows.
Host work is limited to slicing/transposing/casting inputs and concatenating
outputs.

On-device layout: scoresT [keys, queries] so softmax-exp'd probabilities feed
attn@v matmuls directly as the moving operand.

The kernel is softmax-exp bound: the Scalar engine must evaluate
H*SQ*S = 16.8M exps per core (~1.1us per [128,1024] block, 128 blocks).
Everything else is scheduled around keeping that pipeline saturated:

- The matmul path runs in bf16 (1 PE cycle/row vs ~4 for fp32); PSUM
  accumulation stays fp32.
- Softmax denominators ride along in the attn@v matmuls: the V stationary
  carries a 65th column of ones, so row 64 of each accumulator is the
  denominator for free.
- Denominator rows are DMA'd from PSUM partition 64 to SBUF partition 0
  (engines cannot shift partitions; DMA can), reciprocal'd there, then
  partition-broadcast on the otherwise-idle GPSIMD engine (whose ucode
  requires a partition-0 source); normalization is then a plain multiply.
- The attn output halves are assembled into one [128,512] tile via an
  SBUF->SBUF DMA partition shift so o-proj contracts over all 128 dims.
- x/k/v/q tensors are tiled per chunk so dependencies are fine-grained, and
  all projection + o-proj work that is not needed immediately is drained
  1-2 items per kb slot into the PE idle gaps of the exp-bound attention
  loops ("deferred work"), instead of running as serial phases.
"""

import numpy as np

B, S, D = 4, 2048, 512
H, KV, DH = 8, 2, 64
SQ = S // 2  # queries per core
NCORES = 8
PAIRS = 4  # head pairs (p, p+4); p -> kv0 rows 0:64, p+4 -> kv1 rows 64:128
SCALE = 1.0 / 8.0  # 1/sqrt(DH)
PERM = [0, 4, 1, 5, 2, 6, 3, 7]  # q head order: pair-major
NKB = S // 128  # 16 key blocks
NSC = S // 512  # 4 column chunks of x

_built = {}


def _build_nc():
    import concourse.mybir as mybir
    import concourse.tile as tile
    from concourse import bacc

    fp32 = mybir.dt.float32
    bf16 = mybir.dt.bfloat16
    Exp = mybir.ActivationFunctionType.Exp

    nc = bacc.Bacc("TRN2", target_bir_lowering=False, debug=False,
                   num_devices=NCORES)

    # all matrices arrive pre-arranged on the host into the exact SBUF
    # layout [partition, chunk, col] so every input DMA is fully contiguous
    xp = nc.dram_tensor("xp", [128, NSC * 4 * 512], bf16,
                        kind="ExternalInput").ap()
    wq = nc.dram_tensor("wq", [128, 4 * D], bf16, kind="ExternalInput").ap()
    wk = nc.dram_tensor("wk", [128, 4 * 128], bf16, kind="ExternalInput").ap()
    wv = nc.dram_tensor("wv", [128, 4 * 128], bf16, kind="ExternalInput").ap()
    wo = nc.dram_tensor("wo", [128, 4 * D], bf16, kind="ExternalInput").ap()
    bqp = nc.dram_tensor("bqp", [128, PAIRS], fp32, kind="ExternalInput").ap()
    bkvp = nc.dram_tensor("bkvp", [128, 1], fp32, kind="ExternalInput").ap()
    bvbc = nc.dram_tensor("bvbc", [128, 128], fp32, kind="ExternalInput").ap()
    bobc = nc.dram_tensor("bobc", [128, D], fp32, kind="ExternalInput").ap()
    y = nc.dram_tensor("y", [SQ, D], fp32, kind="ExternalOutput").ap()

    with tile.TileContext(nc) as tc:
        with (
            tc.tile_pool(name="consts", bufs=1) as consts,
            tc.tile_pool(name="epool", bufs=4) as epool,
            tc.tile_pool(name="opool", bufs=9) as opool,
            tc.tile_pool(name="obpool", bufs=3) as obpool,
            tc.tile_pool(name="cpool", bufs=2) as cpool,
            tc.tile_pool(name="npool", bufs=3) as npool,
            tc.tile_pool(name="bcpool", bufs=4) as bcpool,
            tc.tile_pool(name="ypool", bufs=3) as ypool,
            tc.tile_pool(name="pssc", bufs=2, space="PSUM") as pssc,
            tc.tile_pool(name="pacc", bufs=2, space="PSUM") as pacc,
            tc.tile_pool(name="pproj", bufs=2, space="PSUM") as pproj,
        ):
            # ---- input DMAs: split across the two HWDGE queues (SP and
            # Activation -- the scalar engine is idle during the prologue) so
            # transfers overlap; everything contiguous via host prearrange.
            # Sync queue carries what gates the first scores (wk, x0, x1);
            # the scalar queue carries wq first, then the rest.
            wk_sb = consts.tile([128, 4, 128], bf16, tag="wk")
            nc.sync.dma_start(wk_sb[:], wk.rearrange("p (c j) -> p c j", c=4))
            xt_ch = []
            for sc in range(NSC):
                xch = consts.tile([128, 4, 512], bf16, name=f"xch{sc}",
                                  tag=f"xt{sc}")
                xt_ch.append(xch)
            nc.sync.dma_start(xt_ch[0][:],
                              xp[:, 0:2048].rearrange("p (c j) -> p c j", c=4))
            nc.sync.dma_start(xt_ch[1][:],
                              xp[:, 2048:4096].rearrange("p (c j) -> p c j", c=4))
            bq_sb = consts.tile([128, PAIRS], fp32, tag="bq")
            nc.sync.dma_start(bq_sb[:], bqp)
            bkv_sb = consts.tile([128, 1], fp32, tag="bkv")
            nc.sync.dma_start(bkv_sb[:], bkvp)
            wq_sb = consts.tile([128, 4, D], bf16, tag="wq")
            nc.scalar.dma_start(wq_sb[:], wq.rearrange("p (c j) -> p c j", c=4))
            wv_sb = consts.tile([128, 4, 128], bf16, tag="wv")
            nc.scalar.dma_start(wv_sb[:], wv.rearrange("p (c j) -> p c j", c=4))
            bv_sb = consts.tile([128, 128], fp32, tag="bv")
            nc.scalar.dma_start(bv_sb[:], bvbc)
            nc.scalar.dma_start(xt_ch[2][:],
                                xp[:, 4096:6144].rearrange("p (c j) -> p c j", c=4))
            nc.scalar.dma_start(xt_ch[3][:],
                                xp[:, 6144:8192].rearrange("p (c j) -> p c j", c=4))
            wo_sb = consts.tile([128, 4, D], bf16, tag="wo")
            nc.scalar.dma_start(wo_sb[:], wo.rearrange("p (c j) -> p c j", c=4))
            bo_sb = consts.tile([128, D], fp32, tag="bo")
            nc.scalar.dma_start(bo_sb[:], bobc)

            # per-chunk kT tiles, per-block V tiles, per-(pair, half) qT
            # tiles so consumers wait only on the piece they need
            ktt = [consts.tile([128, 512], bf16, name=f"ktt{sc}",
                               tag=f"kt{sc}") for sc in range(NSC)]
            # V block: cols 0:64 = v_kv0, 64 = ones, 65:129 = v_kv1, 129 = ones
            vpt = [consts.tile([128, 130], bf16, name=f"vpt{sb}",
                               tag=f"vp{sb}") for sb in range(NKB)]
            qtt = [[consts.tile([128, 512], bf16, name=f"qtt{pr}_{qc}",
                                tag=f"qt{pr}_{qc}") for qc in range(2)]
                   for pr in range(PAIRS)]

            # Projection / o-proj emitters, split into <=2-matmul pieces so
            # they can drain into the short PE idle gaps of the exp-bound
            # attention loops without starving the scalar engine.  Each
            # chain allocates its own pproj PSUM tile in its first piece.
            def kt_mm(sc, cs, box):
                if "ps" not in box:
                    box["ps"] = pproj.tile([128, 512], fp32, name=f"pk{sc}",
                                           tag="pproj")
                ps = box["ps"]
                for c in (cs, cs + 1):
                    nc.tensor.matmul(ps[:], wk_sb[:, c, :],
                                     xt_ch[sc][:, c, :],
                                     start=(c == 0), stop=(c == 3))
                if cs == 2:
                    nc.vector.tensor_scalar_add(ktt[sc][:], ps[:],
                                                bkv_sb[:, 0:1])

            def v_mm(sb, cs, box):
                if "ps" not in box:
                    box["ps"] = pproj.tile([128, 512], fp32, name=f"pv{sb}",
                                           tag="pproj")
                ps = box["ps"]
                xch = xt_ch[sb // 4]
                off = (sb % 4) * 128
                for c in (cs, cs + 1):
                    nc.tensor.matmul(ps[:, 0:128],
                                     xch[:, c, off:off + 128],
                                     wv_sb[:, c, :],
                                     start=(c == 0), stop=(c == 3))
                if cs == 2:
                    nc.vector.memset(vpt[sb][:, 64:65], 1.0)
                    nc.vector.memset(vpt[sb][:, 129:130], 1.0)
                    nc.vector.tensor_add(vpt[sb][:, 0:64], ps[:, 0:64],
                                         bv_sb[:, 0:64])
                    nc.vector.tensor_add(vpt[sb][:, 65:129], ps[:, 64:128],
                                         bv_sb[:, 64:128])

            def qt_mm(pr, qc, cs, box):
                if "ps" not in box:
                    box["ps"] = pproj.tile([128, 512], fp32, name=f"pq{pr}{qc}",
                                           tag="pproj")
                ps = box["ps"]
                for c in (cs, cs + 1):
                    nc.tensor.matmul(ps[:],
                                     wq_sb[:, c, pr * 128:(pr + 1) * 128],
                                     xt_ch[qc][:, c, :],
                                     start=(c == 0), stop=(c == 3))
                if cs == 2:
                    nc.vector.tensor_scalar_add(qtt[pr][qc][:], ps[:],
                                                bq_sb[:, pr:pr + 1])

            ot_tiles = {}  # (qc, pr) -> assembled [128, 512] bf16 attn out

            def oproj_mm(qc, m, prs, box):
                if "ps" not in box:
                    box["ps"] = pproj.tile([128, 512], fp32, name=f"po{qc}{m}",
                                           tag="pproj")
                ps = box["ps"]
                for pr in (prs, prs + 1):
                    nc.tensor.matmul(ps[:],
                                     ot_tiles[(qc, pr)][:, m * 128:(m + 1) * 128],
                                     wo_sb[:, pr, :],
                                     start=(pr == 0), stop=(pr == 3))

            def oproj_fin(qc, m, box):
                yt = ypool.tile([128, 512], fp32, name=f"yt{qc}{m}", tag="y")
                nc.vector.tensor_add(yt[:], box["ps"][:], bo_sb[:])
                blk = qc * 4 + m
                nc.sync.dma_start(y[blk * 128:(blk + 1) * 128, :], yt[:])

            def chain(fn, *idx):
                # full 4-matmul chain emitted inline (prologue / tail)
                box = {}
                fn(*idx, 0, box)
                fn(*idx, 2, box)
                return box

            # ---- serial prologue: the minimum before exp can start ----
            chain(kt_mm, 0)       # kT chunk 0   (wk + x0)
            chain(qt_mm, 0, 0)    # qT pair0 half0  (wq + x0)
            chain(v_mm, 0)        # V blocks 0-2 (attnv is 2 deep)
            chain(v_mm, 1)
            chain(v_mm, 2)

            # deferred 2-matmul pieces drained into the attention loops:
            # {job: {slot: [closure, ...]}}
            deferred = {j: {} for j in range(8)}
            boxes = {}

            def defer(j, slot, key, fn, *idx):
                box = boxes.setdefault(key, {})
                deferred[j].setdefault(slot, []).append(
                    (lambda b: (lambda: fn(*idx, b)))(box))

            # job 0: remaining V blocks (vp(k) needed by attnv(k) at slot
            # k+2) and kT chunks 1-3 (ktt[c] needed by scores at slot 4c)
            defer(0, 0, "k1", kt_mm, 1, 0)
            defer(0, 0, "k1", kt_mm, 1, 2)
            defer(0, 1, "v3", v_mm, 3, 0)
            defer(0, 1, "v3", v_mm, 3, 2)
            vslot = {4: 2, 5: 3, 6: 4, 7: 6, 8: 7, 9: 8, 10: 10, 11: 11,
                     12: 12, 13: 13, 14: 14, 15: 15}
            for k in range(4, NKB):
                defer(0, vslot[k], f"v{k}", v_mm, k, 0)
                defer(0, vslot[k], f"v{k}", v_mm, k, 2)
            defer(0, 5, "k2", kt_mm, 2, 0)
            defer(0, 5, "k2", kt_mm, 2, 2)
            defer(0, 9, "k3", kt_mm, 3, 0)
            defer(0, 9, "k3", kt_mm, 3, 2)
            # pair 0's second half is needed by job 1
            defer(0, 15, "q01", qt_mm, 0, 1, 0)
            defer(0, 15, "q01", qt_mm, 0, 1, 2)
            # qT for the next pair drains across the two jobs before it
            for pr in range(1, PAIRS):
                j = 2 * pr - 2
                defer(j + 1, 4, f"q{pr}0", qt_mm, pr, 0, 0)
                defer(j + 1, 5, f"q{pr}0", qt_mm, pr, 0, 2)
                defer(j + 1, 8, f"q{pr}1", qt_mm, pr, 1, 0)
                defer(j + 1, 9, f"q{pr}1", qt_mm, pr, 1, 2)
            # o-proj for qc0 hides in the last job (ot(0,p3) lands ~slot 5)
            oslots = [(5, 6, 7), (8, 9, 10), (11, 12, 13), (13, 14, 15)]
            for m in range(4):
                a, b, f = oslots[m]
                defer(7, a, f"o{m}", oproj_mm, 0, m, 0)
                defer(7, b, f"o{m}", oproj_mm, 0, m, 2)
                defer(7, f, f"o{m}", oproj_fin, 0, m)

            # ---- 8 attention jobs: qc-major within pair so each pair's qT
            # is reused by consecutive jobs ----
            jobs = [(qc, pr) for pr in range(PAIRS) for qc in range(2)]
            for j, (qc, pr) in enumerate(jobs):
                pA = pacc.tile([128, 512], fp32, tag="pacc")
                pB = pacc.tile([128, 512], fp32, tag="pacc")
                e_tiles = [None] * NKB

                def attnv(kb):
                    e = e_tiles[kb]
                    nc.tensor.matmul(pA[0:65, :], vpt[kb][:, 0:65],
                                     e[:, 0:512],
                                     start=(kb == 0), stop=(kb == NKB - 1))
                    nc.tensor.matmul(pB[0:65, :], vpt[kb][:, 65:130],
                                     e[:, 512:1024],
                                     start=(kb == 0), stop=(kb == NKB - 1))

                for kb in range(NKB):
                    sc_ps = pssc.tile([128, 1024], fp32, tag="scores")
                    nc.tensor.matmul(
                        sc_ps[:, 0:512],
                        ktt[kb // 4][0:64, (kb % 4) * 128:(kb % 4 + 1) * 128],
                        qtt[pr][qc][0:64, :])
                    nc.tensor.matmul(
                        sc_ps[:, 512:1024],
                        ktt[kb // 4][64:128, (kb % 4) * 128:(kb % 4 + 1) * 128],
                        qtt[pr][qc][64:128, :])
                    e = epool.tile([128, 1024], bf16, tag="E")
                    e_tiles[kb] = e
                    nc.scalar.activation(e[:], sc_ps[:], Exp, scale=SCALE)
                    # consume probs two blocks back so PE never waits on exp
                    if kb >= 2:
                        attnv(kb - 2)
                    for fn in deferred[j].get(kb, ()):
                        fn()
                attnv(NKB - 2)
                attnv(NKB - 1)

                # normalize: den_p in pA row 64, den_p+4 in pB row 64.  Copy
                # the live rows to SBUF immediately so the PSUM accumulators
                # free for the next job (pacc is single-buffered), then DMA
                # the den rows to partition 0 (engines cannot shift
                # partitions; the gpsimd broadcast ucode reads partition 0).
                cpA = cpool.tile([65, 512], fp32, tag="cpA")
                cpB = cpool.tile([65, 512], fp32, tag="cpB")
                nc.vector.tensor_copy(cpA[:], pA[0:65, :])
                nc.vector.tensor_copy(cpB[:], pB[0:65, :])
                d0 = npool.tile([1, 1024], fp32, tag="den0")
                nc.sync.dma_start(d0[0:1, 0:512], cpA[64:65, :])
                nc.sync.dma_start(d0[0:1, 512:1024], cpB[64:65, :])
                r0 = npool.tile([1, 1024], fp32, tag="rden0")
                s0 = npool.tile([1, 1024], fp32, tag="rscr0")
                nc.vector.reciprocal_approx_accurate(r0[:], d0[:], s0[:])
                rbcA = bcpool.tile([64, 512], fp32, tag="rbcA")
                rbcB = bcpool.tile([64, 512], fp32, tag="rbcB")
                nc.gpsimd.partition_broadcast(rbcA[:], r0[0:1, 0:512],
                                              channels=64)
                nc.gpsimd.partition_broadcast(rbcB[:], r0[0:1, 512:1024],
                                              channels=64)
                # assemble both normalized halves into one [128, 512] tile
                # (second half via DMA partition shift) so o-proj contracts
                # over all 128 dims at once
                ot = opool.tile([128, 512], bf16, tag="ot")
                nc.vector.tensor_mul(ot[0:64, :], cpA[0:64, :], rbcA[:])
                obt = obpool.tile([64, 512], bf16, tag="obt")
                nc.vector.tensor_mul(obt[:], cpB[0:64, :], rbcB[:])
                nc.sync.dma_start(ot[64:128, :], obt[:])
                ot_tiles[(qc, pr)] = ot

            # tail: o-proj for qc1 (qc0's was drained into job 7).  The
            # pr=0..2 partials only need ots that are long ready, so they
            # run while job 7's normalize chain produces ot(1,p3); only the
            # final pr=3 matmuls wait on it.
            tb = {}
            for m in range(4):
                tb[m] = chain_b = {}
                if m >= 2:
                    chain_b["ps"] = pacc.tile([128, 512], fp32,
                                              name=f"ypt{m}", tag="pacc")
                oproj_mm(1, m, 0, chain_b)  # pr 0,1
            for m in range(4):
                ps = tb[m]["ps"]
                nc.tensor.matmul(ps[:],
                                 ot_tiles[(1, 2)][:, m * 128:(m + 1) * 128],
                                 wo_sb[:, 2, :], start=False, stop=False)
                nc.tensor.matmul(ps[:],
                                 ot_tiles[(1, 3)][:, m * 128:(m + 1) * 128],
                                 wo_sb[:, 3, :], start=False, stop=True)
                oproj_fin(1, m, tb[m])

    nc.finalize()
    return nc


def _get_nc():
    if "nc" not in _built:
        _built["nc"] = _build_nc()
    return _built["nc"]


def _in_maps(x, Wq, bq, Wk, bk, Wv, bv, Wo, bo):
    import ml_dtypes

    b16 = ml_dtypes.bfloat16
    x = np.ascontiguousarray(np.asarray(x, np.float32))
    Wq = np.asarray(Wq, np.float32)
    bq = np.asarray(bq, np.float32)
    Wk = np.asarray(Wk, np.float32)
    bk = np.asarray(bk, np.float32)
    Wv = np.asarray(Wv, np.float32)
    bv = np.asarray(bv, np.float32)
    Wo = np.asarray(Wo, np.float32)
    bo = np.asarray(bo, np.float32)

    def chunked(a):  # [D, n] row-major -> [128, 4*n] with row d = (c, p)
        n = a.shape[1]
        return np.ascontiguousarray(
            a.reshape(4, 128, n).transpose(1, 0, 2).reshape(128, 4 * n))

    wq_p = chunked(
        Wq.reshape(D, H, DH)[:, PERM, :].reshape(D, D)).astype(b16)
    wo_p = chunked(
        Wo.reshape(H, DH, D)[PERM].reshape(D, D)).astype(b16)
    wk_p = chunked(Wk).astype(b16)
    wv_p = chunked(Wv).astype(b16)
    bq_p = np.ascontiguousarray(
        bq.reshape(H, DH)[PERM].reshape(PAIRS, 128).T)
    bkv_p = np.ascontiguousarray(bk.reshape(128, 1))
    bv_bc = np.ascontiguousarray(np.tile(bv[None, :], (128, 1)))
    bo_bc = np.ascontiguousarray(np.tile(bo[None, :], (128, 1)))

    in_maps = []
    for c in range(NCORES):
        b, sh = divmod(c, 2)
        xroll = np.roll(x[b], -sh * SQ, axis=0)
        # xT [D, S] -> [128, sc, c, 512] chunk-major contiguous
        xprep = np.ascontiguousarray(
            xroll.T.reshape(4, 128, 4, 512).transpose(1, 2, 0, 3)
            .reshape(128, NSC * 4 * 512)).astype(b16)
        in_maps.append({
            "xp": xprep,
            "wq": wq_p, "wk": wk_p, "wv": wv_p, "wo": wo_p,
            "bqp": bq_p, "bkvp": bkv_p, "bvbc": bv_bc, "bobc": bo_bc,
        })
    return in_maps


def kernel(x, Wq, bq, Wk, bk, Wv, bv, Wo, bo):
    from concourse.bass_utils import run_bass_kernel_spmd

    in_maps = _in_maps(x, Wq, bq, Wk, bk, Wv, bv, Wo, bo)
    nc = _get_nc()
    res = run_bass_kernel_spmd(nc, in_maps, list(range(NCORES)))
    out = np.empty((B, S, D), np.float32)
    for c in range(NCORES):
        b, sh = divmod(c, 2)
        out[b, sh * SQ:(sh + 1) * SQ, :] = res.results[c]["y"]
    return out


# revision 22
# speedup vs baseline: 1.1745x; 1.0169x over previous
"""GroupedQueryAttention kernel for 8 Trainium2 NeuronCores.

Sharding: core c = (batch b = c//2, seq-half sh = c%2). Each core computes the
full attention output for 1024 query rows of one batch: all 8 q heads
(2 kv heads), plus the q/k/v projections and the o-projection for those rows.
Host work is limited to slicing/transposing/casting inputs and concatenating
outputs.

On-device layout: scoresT [keys, queries] so softmax-exp'd probabilities feed
attn@v matmuls directly as the moving operand.

The kernel is softmax-exp bound: the Scalar engine must evaluate
H*SQ*S = 16.8M exps per core (~1.1us per [128,1024] block, 128 blocks).
Everything else is scheduled around keeping that pipeline saturated:

- The matmul path runs in bf16 (1 PE cycle/row vs ~4 for fp32); PSUM
  accumulation stays fp32.
- Softmax denominators ride along in the attn@v matmuls: the V stationary
  carries a 65th column of ones, so row 64 of each accumulator is the
  denominator for free.
- Denominator rows are DMA'd from PSUM partition 64 to SBUF partition 0
  (engines cannot shift partitions; DMA can), reciprocal'd there, then
  partition-broadcast on the otherwise-idle GPSIMD engine (whose ucode
  requires a partition-0 source); normalization is then a plain multiply.
- The attn output halves are assembled into one [128,512] tile via an
  SBUF->SBUF DMA partition shift so o-proj contracts over all 128 dims.
- x/k/v/q tensors are tiled per chunk so dependencies are fine-grained, and
  all projection + o-proj work that is not needed immediately is drained
  1-2 items per kb slot into the PE idle gaps of the exp-bound attention
  loops ("deferred work"), instead of running as serial phases.
"""

import numpy as np

B, S, D = 4, 2048, 512
H, KV, DH = 8, 2, 64
SQ = S // 2  # queries per core
NCORES = 8
PAIRS = 4  # head pairs (p, p+4); p -> kv0 rows 0:64, p+4 -> kv1 rows 64:128
SCALE = 1.0 / 8.0  # 1/sqrt(DH)
PERM = [0, 4, 1, 5, 2, 6, 3, 7]  # q head order: pair-major
NKB = S // 128  # 16 key blocks
NSC = S // 512  # 4 column chunks of x

_built = {}


def _build_nc():
    import concourse.mybir as mybir
    import concourse.tile as tile
    from concourse import bacc

    fp32 = mybir.dt.float32
    bf16 = mybir.dt.bfloat16
    Exp = mybir.ActivationFunctionType.Exp

    nc = bacc.Bacc("TRN2", target_bir_lowering=False, debug=False,
                   num_devices=NCORES)

    # all matrices arrive pre-arranged on the host into the exact SBUF
    # layout [partition, chunk, col] so every input DMA is fully contiguous
    xp = nc.dram_tensor("xp", [128, NSC * 4 * 512], bf16,
                        kind="ExternalInput").ap()
    wq = nc.dram_tensor("wq", [128, 4 * D], bf16, kind="ExternalInput").ap()
    wk = nc.dram_tensor("wk", [128, 4 * 128], bf16, kind="ExternalInput").ap()
    wv = nc.dram_tensor("wv", [128, 4 * 128], bf16, kind="ExternalInput").ap()
    wo = nc.dram_tensor("wo", [128, 4 * D], bf16, kind="ExternalInput").ap()
    bqp = nc.dram_tensor("bqp", [128, PAIRS], fp32, kind="ExternalInput").ap()
    bkvp = nc.dram_tensor("bkvp", [128, 1], fp32, kind="ExternalInput").ap()
    bvbc = nc.dram_tensor("bvbc", [128, 128], fp32, kind="ExternalInput").ap()
    bobc = nc.dram_tensor("bobc", [128, D], fp32, kind="ExternalInput").ap()
    y = nc.dram_tensor("y", [SQ, D], fp32, kind="ExternalOutput").ap()

    with tile.TileContext(nc) as tc:
        with (
            tc.tile_pool(name="consts", bufs=1) as consts,
            tc.tile_pool(name="epool", bufs=4) as epool,
            tc.tile_pool(name="opool", bufs=9) as opool,
            tc.tile_pool(name="obpool", bufs=3) as obpool,
            tc.tile_pool(name="cpool", bufs=2) as cpool,
            tc.tile_pool(name="npool", bufs=3) as npool,
            tc.tile_pool(name="bcpool", bufs=4) as bcpool,
            tc.tile_pool(name="ypool", bufs=3) as ypool,
            tc.tile_pool(name="pssc", bufs=2, space="PSUM") as pssc,
            tc.tile_pool(name="pacc", bufs=2, space="PSUM") as pacc,
            tc.tile_pool(name="pproj", bufs=2, space="PSUM") as pproj,
        ):
            # ---- input DMAs: split across the two HWDGE queues (SP and
            # Activation -- the scalar engine is idle during the prologue) so
            # transfers overlap; everything contiguous via host prearrange.
            # Sync queue carries what gates the first scores (wk, x0, x1);
            # the scalar queue carries wq first, then the rest.
            wk_sb = consts.tile([128, 4, 128], bf16, tag="wk")
            nc.sync.dma_start(wk_sb[:], wk.rearrange("p (c j) -> p c j", c=4))
            xt_ch = []
            for sc in range(NSC):
                xch = consts.tile([128, 4, 512], bf16, name=f"xch{sc}",
                                  tag=f"xt{sc}")
                xt_ch.append(xch)
            nc.sync.dma_start(xt_ch[0][:],
                              xp[:, 0:2048].rearrange("p (c j) -> p c j", c=4))
            nc.sync.dma_start(xt_ch[1][:],
                              xp[:, 2048:4096].rearrange("p (c j) -> p c j", c=4))
            bq_sb = consts.tile([128, PAIRS], fp32, tag="bq")
            nc.sync.dma_start(bq_sb[:], bqp)
            bkv_sb = consts.tile([128, 1], fp32, tag="bkv")
            nc.sync.dma_start(bkv_sb[:], bkvp)
            wq_sb = consts.tile([128, 4, D], bf16, tag="wq")
            nc.scalar.dma_start(wq_sb[:], wq.rearrange("p (c j) -> p c j", c=4))
            wv_sb = consts.tile([128, 4, 128], bf16, tag="wv")
            nc.scalar.dma_start(wv_sb[:], wv.rearrange("p (c j) -> p c j", c=4))
            bv_sb = consts.tile([128, 128], fp32, tag="bv")
            nc.scalar.dma_start(bv_sb[:], bvbc)
            nc.scalar.dma_start(xt_ch[2][:],
                                xp[:, 4096:6144].rearrange("p (c j) -> p c j", c=4))
            nc.scalar.dma_start(xt_ch[3][:],
                                xp[:, 6144:8192].rearrange("p (c j) -> p c j", c=4))
            wo_sb = consts.tile([128, 4, D], bf16, tag="wo")
            nc.scalar.dma_start(wo_sb[:], wo.rearrange("p (c j) -> p c j", c=4))
            bo_sb = consts.tile([128, D], fp32, tag="bo")
            nc.scalar.dma_start(bo_sb[:], bobc)

            # per-chunk kT tiles, per-block V tiles, per-(pair, half) qT
            # tiles so consumers wait only on the piece they need
            ktt = [consts.tile([128, 512], bf16, name=f"ktt{sc}",
                               tag=f"kt{sc}") for sc in range(NSC)]
            # V block: cols 0:64 = v_kv0, 64 = ones, 65:129 = v_kv1, 129 = ones
            vpt = [consts.tile([128, 130], bf16, name=f"vpt{sb}",
                               tag=f"vp{sb}") for sb in range(NKB)]
            qtt = [[consts.tile([128, 512], bf16, name=f"qtt{pr}_{qc}",
                                tag=f"qt{pr}_{qc}") for qc in range(2)]
                   for pr in range(PAIRS)]

            # Projection / o-proj emitters, split into <=2-matmul pieces so
            # they can drain into the short PE idle gaps of the exp-bound
            # attention loops without starving the scalar engine.  Each
            # chain allocates its own pproj PSUM tile in its first piece.
            def kt_mm(sc, cs, box):
                if "ps" not in box:
                    box["ps"] = pproj.tile([128, 512], fp32, name=f"pk{sc}",
                                           tag="pproj")
                ps = box["ps"]
                for c in (cs, cs + 1):
                    nc.tensor.matmul(ps[:], wk_sb[:, c, :],
                                     xt_ch[sc][:, c, :],
                                     start=(c == 0), stop=(c == 3))
                if cs == 2:
                    nc.vector.tensor_scalar_add(ktt[sc][:], ps[:],
                                                bkv_sb[:, 0:1])

            def v_mm(sb, cs, box):
                if "ps" not in box:
                    box["ps"] = pproj.tile([128, 512], fp32, name=f"pv{sb}",
                                           tag="pproj")
                ps = box["ps"]
                xch = xt_ch[sb // 4]
                off = (sb % 4) * 128
                for c in (cs, cs + 1):
                    nc.tensor.matmul(ps[:, 0:128],
                                     xch[:, c, off:off + 128],
                                     wv_sb[:, c, :],
                                     start=(c == 0), stop=(c == 3))
                if cs == 2:
                    nc.vector.memset(vpt[sb][:, 64:65], 1.0)
                    nc.vector.memset(vpt[sb][:, 129:130], 1.0)
                    nc.vector.tensor_add(vpt[sb][:, 0:64], ps[:, 0:64],
                                         bv_sb[:, 0:64])
                    nc.vector.tensor_add(vpt[sb][:, 65:129], ps[:, 64:128],
                                         bv_sb[:, 64:128])

            def qt_mm(pr, qc, cs, box):
                if "ps" not in box:
                    box["ps"] = pproj.tile([128, 512], fp32, name=f"pq{pr}{qc}",
                                           tag="pproj")
                ps = box["ps"]
                for c in (cs, cs + 1):
                    nc.tensor.matmul(ps[:],
                                     wq_sb[:, c, pr * 128:(pr + 1) * 128],
                                     xt_ch[qc][:, c, :],
                                     start=(c == 0), stop=(c == 3))
                if cs == 2:
                    nc.vector.tensor_scalar_add(qtt[pr][qc][:], ps[:],
                                                bq_sb[:, pr:pr + 1])

            ot_tiles = {}  # (qc, pr) -> assembled [128, 512] bf16 attn out

            def oproj_mm(qc, m, prs, box):
                if "ps" not in box:
                    box["ps"] = pproj.tile([128, 512], fp32, name=f"po{qc}{m}",
                                           tag="pproj")
                ps = box["ps"]
                for pr in (prs, prs + 1):
                    nc.tensor.matmul(ps[:],
                                     ot_tiles[(qc, pr)][:, m * 128:(m + 1) * 128],
                                     wo_sb[:, pr, :],
                                     start=(pr == 0), stop=(pr == 3))

            def oproj_fin(qc, m, box):
                yt = ypool.tile([128, 512], fp32, name=f"yt{qc}{m}", tag="y")
                nc.vector.tensor_add(yt[:], box["ps"][:], bo_sb[:])
                blk = qc * 4 + m
                eng = nc.sync if m % 2 == 0 else nc.scalar
                eng.dma_start(y[blk * 128:(blk + 1) * 128, :], yt[:])

            def chain(fn, *idx):
                # full 4-matmul chain emitted inline (prologue / tail)
                box = {}
                fn(*idx, 0, box)
                fn(*idx, 2, box)
                return box

            # ---- serial prologue: the minimum before exp can start ----
            chain(kt_mm, 0)       # kT chunk 0   (wk + x0)
            chain(qt_mm, 0, 0)    # qT pair0 half0  (wq + x0)
            chain(v_mm, 0)        # V block 0 (attnv is 2 deep; v1/v2
                                  # drain in job-0 slots 0/1)

            # deferred 2-matmul pieces drained into the attention loops:
            # {job: {slot: [closure, ...]}}
            deferred = {j: {} for j in range(8)}
            boxes = {}

            def defer(j, slot, key, fn, *idx):
                box = boxes.setdefault(key, {})
                deferred[j].setdefault(slot, []).append(
                    (lambda b: (lambda: fn(*idx, b)))(box))

            # job 0: remaining V blocks (vp(k) needed by attnv(k) at slot
            # k+2) and kT chunks 1-3 (ktt[c] needed by scores at slot 4c)
            defer(0, 0, "v1", v_mm, 1, 0)
            defer(0, 0, "v1", v_mm, 1, 2)
            defer(0, 0, "k1", kt_mm, 1, 0)
            defer(0, 0, "k1", kt_mm, 1, 2)
            defer(0, 1, "v2", v_mm, 2, 0)
            defer(0, 1, "v2", v_mm, 2, 2)
            defer(0, 1, "v3", v_mm, 3, 0)
            defer(0, 1, "v3", v_mm, 3, 2)
            vslot = {4: 2, 5: 3, 6: 4, 7: 6, 8: 7, 9: 8, 10: 10, 11: 11,
                     12: 12, 13: 13, 14: 14, 15: 15}
            for k in range(4, NKB):
                defer(0, vslot[k], f"v{k}", v_mm, k, 0)
                defer(0, vslot[k], f"v{k}", v_mm, k, 2)
            defer(0, 5, "k2", kt_mm, 2, 0)
            defer(0, 5, "k2", kt_mm, 2, 2)
            defer(0, 9, "k3", kt_mm, 3, 0)
            defer(0, 9, "k3", kt_mm, 3, 2)
            # pair 0's second half is needed by job 1
            defer(0, 15, "q01", qt_mm, 0, 1, 0)
            defer(0, 15, "q01", qt_mm, 0, 1, 2)
            # qT for the next pair drains across the two jobs before it
            for pr in range(1, PAIRS):
                j = 2 * pr - 2
                defer(j + 1, 4, f"q{pr}0", qt_mm, pr, 0, 0)
                defer(j + 1, 5, f"q{pr}0", qt_mm, pr, 0, 2)
                defer(j + 1, 8, f"q{pr}1", qt_mm, pr, 1, 0)
                defer(j + 1, 9, f"q{pr}1", qt_mm, pr, 1, 2)
            # o-proj for qc0 hides in the last job (ot(0,p3) lands ~slot 5)
            oslots = [(5, 6, 7), (8, 9, 10), (11, 12, 13), (13, 14, 15)]
            for m in range(4):
                a, b, f = oslots[m]
                defer(7, a, f"o{m}", oproj_mm, 0, m, 0)
                defer(7, b, f"o{m}", oproj_mm, 0, m, 2)
                defer(7, f, f"o{m}", oproj_fin, 0, m)

            # ---- 8 attention jobs: qc-major within pair so each pair's qT
            # is reused by consecutive jobs ----
            jobs = [(qc, pr) for pr in range(PAIRS) for qc in range(2)]
            for j, (qc, pr) in enumerate(jobs):
                pA = pacc.tile([128, 512], fp32, tag="pacc")
                pB = pacc.tile([128, 512], fp32, tag="pacc")
                e_tiles = [None] * NKB

                def attnv(kb):
                    e = e_tiles[kb]
                    nc.tensor.matmul(pA[0:65, :], vpt[kb][:, 0:65],
                                     e[:, 0:512],
                                     start=(kb == 0), stop=(kb == NKB - 1))
                    nc.tensor.matmul(pB[0:65, :], vpt[kb][:, 65:130],
                                     e[:, 512:1024],
                                     start=(kb == 0), stop=(kb == NKB - 1))

                for kb in range(NKB):
                    sc_ps = pssc.tile([128, 1024], fp32, tag="scores")
                    nc.tensor.matmul(
                        sc_ps[:, 0:512],
                        ktt[kb // 4][0:64, (kb % 4) * 128:(kb % 4 + 1) * 128],
                        qtt[pr][qc][0:64, :])
                    nc.tensor.matmul(
                        sc_ps[:, 512:1024],
                        ktt[kb // 4][64:128, (kb % 4) * 128:(kb % 4 + 1) * 128],
                        qtt[pr][qc][64:128, :])
                    e = epool.tile([128, 1024], bf16, tag="E")
                    e_tiles[kb] = e
                    nc.scalar.activation(e[:], sc_ps[:], Exp, scale=SCALE)
                    # consume probs two blocks back so PE never waits on exp
                    if kb >= 2:
                        attnv(kb - 2)
                    for fn in deferred[j].get(kb, ()):
                        fn()
                attnv(NKB - 2)
                attnv(NKB - 1)

                # normalize: den_p in pA row 64, den_p+4 in pB row 64.  Copy
                # the live rows to SBUF immediately so the PSUM accumulators
                # free for the next job (pacc is single-buffered), then DMA
                # the den rows to partition 0 (engines cannot shift
                # partitions; the gpsimd broadcast ucode reads partition 0).
                cpA = cpool.tile([65, 512], fp32, tag="cpA")
                cpB = cpool.tile([65, 512], fp32, tag="cpB")
                nc.vector.tensor_copy(cpA[:], pA[0:65, :])
                nc.vector.tensor_copy(cpB[:], pB[0:65, :])
                d0 = npool.tile([1, 1024], fp32, tag="den0")
                nc.sync.dma_start(d0[0:1, 0:512], cpA[64:65, :])
                nc.sync.dma_start(d0[0:1, 512:1024], cpB[64:65, :])
                r0 = npool.tile([1, 1024], fp32, tag="rden0")
                s0 = npool.tile([1, 1024], fp32, tag="rscr0")
                nc.vector.reciprocal_approx_accurate(r0[:], d0[:], s0[:])
                rbcA = bcpool.tile([64, 512], fp32, tag="rbcA")
                rbcB = bcpool.tile([64, 512], fp32, tag="rbcB")
                nc.gpsimd.partition_broadcast(rbcA[:], r0[0:1, 0:512],
                                              channels=64)
                nc.gpsimd.partition_broadcast(rbcB[:], r0[0:1, 512:1024],
                                              channels=64)
                # assemble both normalized halves into one [128, 512] tile
                # (second half via DMA partition shift) so o-proj contracts
                # over all 128 dims at once
                ot = opool.tile([128, 512], bf16, tag="ot")
                nc.vector.tensor_mul(ot[0:64, :], cpA[0:64, :], rbcA[:])
                obt = obpool.tile([64, 512], bf16, tag="obt")
                nc.vector.tensor_mul(obt[:], cpB[0:64, :], rbcB[:])
                nc.sync.dma_start(ot[64:128, :], obt[:])
                ot_tiles[(qc, pr)] = ot

            # tail: o-proj for qc1 (qc0's was drained into job 7).  The
            # pr=0..2 partials only need ots that are long ready, so they
            # run while job 7's normalize chain produces ot(1,p3); only the
            # final pr=3 matmuls wait on it.
            tb = {}
            for m in range(4):
                tb[m] = chain_b = {}
                if m >= 2:
                    chain_b["ps"] = pacc.tile([128, 512], fp32,
                                              name=f"ypt{m}", tag="pacc")
                oproj_mm(1, m, 0, chain_b)  # pr 0,1
            for m in range(4):
                ps = tb[m]["ps"]
                nc.tensor.matmul(ps[:],
                                 ot_tiles[(1, 2)][:, m * 128:(m + 1) * 128],
                                 wo_sb[:, 2, :], start=False, stop=False)
                nc.tensor.matmul(ps[:],
                                 ot_tiles[(1, 3)][:, m * 128:(m + 1) * 128],
                                 wo_sb[:, 3, :], start=False, stop=True)
                oproj_fin(1, m, tb[m])

    nc.finalize()
    return nc


def _get_nc():
    if "nc" not in _built:
        _built["nc"] = _build_nc()
    return _built["nc"]


def _in_maps(x, Wq, bq, Wk, bk, Wv, bv, Wo, bo):
    import ml_dtypes

    b16 = ml_dtypes.bfloat16
    x = np.ascontiguousarray(np.asarray(x, np.float32))
    Wq = np.asarray(Wq, np.float32)
    bq = np.asarray(bq, np.float32)
    Wk = np.asarray(Wk, np.float32)
    bk = np.asarray(bk, np.float32)
    Wv = np.asarray(Wv, np.float32)
    bv = np.asarray(bv, np.float32)
    Wo = np.asarray(Wo, np.float32)
    bo = np.asarray(bo, np.float32)

    def chunked(a):  # [D, n] row-major -> [128, 4*n] with row d = (c, p)
        n = a.shape[1]
        return np.ascontiguousarray(
            a.reshape(4, 128, n).transpose(1, 0, 2).reshape(128, 4 * n))

    wq_p = chunked(
        Wq.reshape(D, H, DH)[:, PERM, :].reshape(D, D)).astype(b16)
    wo_p = chunked(
        Wo.reshape(H, DH, D)[PERM].reshape(D, D)).astype(b16)
    wk_p = chunked(Wk).astype(b16)
    wv_p = chunked(Wv).astype(b16)
    bq_p = np.ascontiguousarray(
        bq.reshape(H, DH)[PERM].reshape(PAIRS, 128).T)
    bkv_p = np.ascontiguousarray(bk.reshape(128, 1))
    bv_bc = np.ascontiguousarray(np.tile(bv[None, :], (128, 1)))
    bo_bc = np.ascontiguousarray(np.tile(bo[None, :], (128, 1)))

    in_maps = []
    for c in range(NCORES):
        b, sh = divmod(c, 2)
        xroll = np.roll(x[b], -sh * SQ, axis=0)
        # xT [D, S] -> [128, sc, c, 512] chunk-major contiguous
        xprep = np.ascontiguousarray(
            xroll.T.reshape(4, 128, 4, 512).transpose(1, 2, 0, 3)
            .reshape(128, NSC * 4 * 512)).astype(b16)
        in_maps.append({
            "xp": xprep,
            "wq": wq_p, "wk": wk_p, "wv": wv_p, "wo": wo_p,
            "bqp": bq_p, "bkvp": bkv_p, "bvbc": bv_bc, "bobc": bo_bc,
        })
    return in_maps


def kernel(x, Wq, bq, Wk, bk, Wv, bv, Wo, bo):
    from concourse.bass_utils import run_bass_kernel_spmd

    in_maps = _in_maps(x, Wq, bq, Wk, bk, Wv, bv, Wo, bo)
    nc = _get_nc()
    res = run_bass_kernel_spmd(nc, in_maps, list(range(NCORES)))
    out = np.empty((B, S, D), np.float32)
    for c in range(NCORES):
        b, sh = divmod(c, 2)
        out[b, sh * SQ:(sh + 1) * SQ, :] = res.results[c]["y"]
    return out


# revision 23
# speedup vs baseline: 1.1774x; 1.0024x over previous
"""GroupedQueryAttention kernel for 8 Trainium2 NeuronCores.

Sharding: core c = (batch b = c//2, seq-half sh = c%2). Each core computes the
full attention output for 1024 query rows of one batch: all 8 q heads
(2 kv heads), plus the q/k/v projections and the o-projection for those rows.
Host work is limited to slicing/transposing/casting inputs and concatenating
outputs.

On-device layout: scoresT [keys, queries] so softmax-exp'd probabilities feed
attn@v matmuls directly as the moving operand.

The kernel is softmax-exp bound: the Scalar engine must evaluate
H*SQ*S = 16.8M exps per core (~1.1us per [128,1024] block, 128 blocks).
Everything else is scheduled around keeping that pipeline saturated:

- The matmul path runs in bf16 (1 PE cycle/row vs ~4 for fp32); PSUM
  accumulation stays fp32.
- Softmax denominators ride along in the attn@v matmuls: the V stationary
  carries a 65th column of ones, so row 64 of each accumulator is the
  denominator for free.
- Denominator rows are DMA'd from PSUM partition 64 to SBUF partition 0
  (engines cannot shift partitions; DMA can), reciprocal'd there, then
  partition-broadcast on the otherwise-idle GPSIMD engine (whose ucode
  requires a partition-0 source); normalization is then a plain multiply.
- The attn output halves are assembled into one [128,512] tile via an
  SBUF->SBUF DMA partition shift so o-proj contracts over all 128 dims.
- x/k/v/q tensors are tiled per chunk so dependencies are fine-grained, and
  all projection + o-proj work that is not needed immediately is drained
  1-2 items per kb slot into the PE idle gaps of the exp-bound attention
  loops ("deferred work"), instead of running as serial phases.
"""

import numpy as np

B, S, D = 4, 2048, 512
H, KV, DH = 8, 2, 64
SQ = S // 2  # queries per core
NCORES = 8
PAIRS = 4  # head pairs (p, p+4); p -> kv0 rows 0:64, p+4 -> kv1 rows 64:128
SCALE = 1.0 / 8.0  # 1/sqrt(DH)
PERM = [0, 4, 1, 5, 2, 6, 3, 7]  # q head order: pair-major
NKB = S // 128  # 16 key blocks
NSC = S // 512  # 4 column chunks of x

_built = {}


def _build_nc():
    import concourse.mybir as mybir
    import concourse.tile as tile
    from concourse import bacc

    fp32 = mybir.dt.float32
    bf16 = mybir.dt.bfloat16
    Exp = mybir.ActivationFunctionType.Exp

    nc = bacc.Bacc("TRN2", target_bir_lowering=False, debug=False,
                   num_devices=NCORES)

    # all matrices arrive pre-arranged on the host into the exact SBUF
    # layout [partition, chunk, col] so every input DMA is fully contiguous
    xp = nc.dram_tensor("xp", [128, NSC * 4 * 512], bf16,
                        kind="ExternalInput").ap()
    wq = nc.dram_tensor("wq", [128, 4 * D], bf16, kind="ExternalInput").ap()
    wk = nc.dram_tensor("wk", [128, 4 * 128], bf16, kind="ExternalInput").ap()
    wv = nc.dram_tensor("wv", [128, 4 * 128], bf16, kind="ExternalInput").ap()
    wo = nc.dram_tensor("wo", [128, 4 * D], bf16, kind="ExternalInput").ap()
    bqp = nc.dram_tensor("bqp", [128, PAIRS], fp32, kind="ExternalInput").ap()
    bkvp = nc.dram_tensor("bkvp", [128, 1], fp32, kind="ExternalInput").ap()
    bvbc = nc.dram_tensor("bvbc", [128, 128], fp32, kind="ExternalInput").ap()
    bobc = nc.dram_tensor("bobc", [128, D], fp32, kind="ExternalInput").ap()
    y = nc.dram_tensor("y", [SQ, D], fp32, kind="ExternalOutput").ap()

    with tile.TileContext(nc) as tc:
        with (
            tc.tile_pool(name="consts", bufs=1) as consts,
            tc.tile_pool(name="epool", bufs=4) as epool,
            tc.tile_pool(name="opool", bufs=9) as opool,
            tc.tile_pool(name="obpool", bufs=3) as obpool,
            tc.tile_pool(name="cpool", bufs=2) as cpool,
            tc.tile_pool(name="npool", bufs=3) as npool,
            tc.tile_pool(name="bcpool", bufs=4) as bcpool,
            tc.tile_pool(name="ypool", bufs=3) as ypool,
            tc.tile_pool(name="pssc", bufs=2, space="PSUM") as pssc,
            tc.tile_pool(name="pacc", bufs=2, space="PSUM") as pacc,
            tc.tile_pool(name="pproj", bufs=2, space="PSUM") as pproj,
        ):
            # ---- input DMAs: split across the two HWDGE queues (SP and
            # Activation -- the scalar engine is idle during the prologue) so
            # transfers overlap; everything contiguous via host prearrange.
            # Sync queue carries what gates the first scores (wk, x0, x1);
            # the scalar queue carries wq first, then the rest.
            wk_sb = consts.tile([128, 4, 128], bf16, tag="wk")
            nc.sync.dma_start(wk_sb[:], wk.rearrange("p (c j) -> p c j", c=4))
            xt_ch = []
            for sc in range(NSC):
                xch = consts.tile([128, 4, 512], bf16, name=f"xch{sc}",
                                  tag=f"xt{sc}")
                xt_ch.append(xch)
            nc.sync.dma_start(xt_ch[0][:],
                              xp[:, 0:2048].rearrange("p (c j) -> p c j", c=4))
            nc.sync.dma_start(xt_ch[1][:],
                              xp[:, 2048:4096].rearrange("p (c j) -> p c j", c=4))
            bq_sb = consts.tile([128, PAIRS], fp32, tag="bq")
            nc.sync.dma_start(bq_sb[:], bqp)
            bkv_sb = consts.tile([128, 1], fp32, tag="bkv")
            nc.sync.dma_start(bkv_sb[:], bkvp)
            wq_sb = consts.tile([128, 4, D], bf16, tag="wq")
            nc.scalar.dma_start(wq_sb[:], wq.rearrange("p (c j) -> p c j", c=4))
            wv_sb = consts.tile([128, 4, 128], bf16, tag="wv")
            nc.scalar.dma_start(wv_sb[:], wv.rearrange("p (c j) -> p c j", c=4))
            bv_sb = consts.tile([128, 128], fp32, tag="bv")
            nc.scalar.dma_start(bv_sb[:], bvbc)
            nc.scalar.dma_start(xt_ch[2][:],
                                xp[:, 4096:6144].rearrange("p (c j) -> p c j", c=4))
            nc.scalar.dma_start(xt_ch[3][:],
                                xp[:, 6144:8192].rearrange("p (c j) -> p c j", c=4))
            wo_sb = consts.tile([128, 4, D], bf16, tag="wo")
            nc.scalar.dma_start(wo_sb[:], wo.rearrange("p (c j) -> p c j", c=4))
            bo_sb = consts.tile([128, D], fp32, tag="bo")
            nc.scalar.dma_start(bo_sb[:], bobc)

            # per-chunk kT tiles, per-block V tiles, per-(pair, half) qT
            # tiles so consumers wait only on the piece they need
            ktt = [consts.tile([128, 512], bf16, name=f"ktt{sc}",
                               tag=f"kt{sc}") for sc in range(NSC)]
            # V block: cols 0:64 = v_kv0, 64 = ones, 65:129 = v_kv1, 129 = ones
            vpt = [consts.tile([128, 130], bf16, name=f"vpt{sb}",
                               tag=f"vp{sb}") for sb in range(NKB)]
            qtt = [[consts.tile([128, 512], bf16, name=f"qtt{pr}_{qc}",
                                tag=f"qt{pr}_{qc}") for qc in range(2)]
                   for pr in range(PAIRS)]

            # Projection / o-proj emitters, split into <=2-matmul pieces so
            # they can drain into the short PE idle gaps of the exp-bound
            # attention loops without starving the scalar engine.  Each
            # chain allocates its own pproj PSUM tile in its first piece.
            def kt_mm(sc, cs, box):
                if "ps" not in box:
                    box["ps"] = pproj.tile([128, 512], fp32, name=f"pk{sc}",
                                           tag="pproj")
                ps = box["ps"]
                for c in (cs, cs + 1):
                    nc.tensor.matmul(ps[:], wk_sb[:, c, :],
                                     xt_ch[sc][:, c, :],
                                     start=(c == 0), stop=(c == 3))
                if cs == 2:
                    nc.vector.tensor_scalar_add(ktt[sc][:], ps[:],
                                                bkv_sb[:, 0:1])

            def v_mm(sb, cs, box):
                if "ps" not in box:
                    box["ps"] = pproj.tile([128, 512], fp32, name=f"pv{sb}",
                                           tag="pproj")
                ps = box["ps"]
                xch = xt_ch[sb // 4]
                off = (sb % 4) * 128
                for c in (cs, cs + 1):
                    nc.tensor.matmul(ps[:, 0:128],
                                     xch[:, c, off:off + 128],
                                     wv_sb[:, c, :],
                                     start=(c == 0), stop=(c == 3))
                if cs == 2:
                    nc.vector.memset(vpt[sb][:, 64:65], 1.0)
                    nc.vector.memset(vpt[sb][:, 129:130], 1.0)
                    nc.vector.tensor_add(vpt[sb][:, 0:64], ps[:, 0:64],
                                         bv_sb[:, 0:64])
                    nc.vector.tensor_add(vpt[sb][:, 65:129], ps[:, 64:128],
                                         bv_sb[:, 64:128])

            def qt_mm(pr, qc, cs, box):
                if "ps" not in box:
                    box["ps"] = pproj.tile([128, 512], fp32, name=f"pq{pr}{qc}",
                                           tag="pproj")
                ps = box["ps"]
                for c in (cs, cs + 1):
                    nc.tensor.matmul(ps[:],
                                     wq_sb[:, c, pr * 128:(pr + 1) * 128],
                                     xt_ch[qc][:, c, :],
                                     start=(c == 0), stop=(c == 3))
                if cs == 2:
                    nc.vector.tensor_scalar_add(qtt[pr][qc][:], ps[:],
                                                bq_sb[:, pr:pr + 1])

            ot_tiles = {}  # (qc, pr) -> assembled [128, 512] bf16 attn out

            def oproj_mm(qc, m, prs, box):
                if "ps" not in box:
                    box["ps"] = pproj.tile([128, 512], fp32, name=f"po{qc}{m}",
                                           tag="pproj")
                ps = box["ps"]
                for pr in (prs, prs + 1):
                    nc.tensor.matmul(ps[:],
                                     ot_tiles[(qc, pr)][:, m * 128:(m + 1) * 128],
                                     wo_sb[:, pr, :],
                                     start=(pr == 0), stop=(pr == 3))

            def oproj_fin(qc, m, box):
                yt = ypool.tile([128, 512], fp32, name=f"yt{qc}{m}", tag="y")
                nc.vector.tensor_add(yt[:], box["ps"][:], bo_sb[:])
                blk = qc * 4 + m
                eng = nc.sync if m % 2 == 0 else nc.scalar
                eng.dma_start(y[blk * 128:(blk + 1) * 128, :], yt[:])

            def chain(fn, *idx):
                # full 4-matmul chain emitted inline (prologue / tail)
                box = {}
                fn(*idx, 0, box)
                fn(*idx, 2, box)
                return box

            # ---- serial prologue: the minimum before exp can start ----
            chain(kt_mm, 0)       # kT chunk 0   (wk + x0)
            chain(qt_mm, 0, 0)    # qT pair0 half0  (wq + x0)
            chain(v_mm, 0)        # V block 0 (attnv is 2 deep; v1/v2
                                  # drain in job-0 slots 0/1)

            # deferred 2-matmul pieces drained into the attention loops:
            # {job: {slot: [closure, ...]}}
            deferred = {j: {} for j in range(8)}
            boxes = {}

            def defer(j, slot, key, fn, *idx):
                box = boxes.setdefault(key, {})
                deferred[j].setdefault(slot, []).append(
                    (lambda b: (lambda: fn(*idx, b)))(box))

            # job 0: remaining V blocks (vp(k) needed by attnv(k) at slot
            # k+2) and kT chunks 1-3 (ktt[c] needed by scores at slot 4c)
            defer(0, 0, "v1", v_mm, 1, 0)
            defer(0, 0, "v1", v_mm, 1, 2)
            defer(0, 0, "k1", kt_mm, 1, 0)
            defer(0, 0, "k1", kt_mm, 1, 2)
            defer(0, 1, "v2", v_mm, 2, 0)
            defer(0, 1, "v2", v_mm, 2, 2)
            defer(0, 1, "v3", v_mm, 3, 0)
            defer(0, 1, "v3", v_mm, 3, 2)
            vslot = {4: 2, 5: 3, 6: 4, 7: 6, 8: 7, 9: 8, 10: 10, 11: 11,
                     12: 12, 13: 13, 14: 14, 15: 15}
            for k in range(4, NKB):
                defer(0, vslot[k], f"v{k}", v_mm, k, 0)
                defer(0, vslot[k], f"v{k}", v_mm, k, 2)
            defer(0, 5, "k2", kt_mm, 2, 0)
            defer(0, 5, "k2", kt_mm, 2, 2)
            defer(0, 9, "k3", kt_mm, 3, 0)
            defer(0, 9, "k3", kt_mm, 3, 2)
            # pair 0's second half is needed by job 1
            defer(0, 15, "q01", qt_mm, 0, 1, 0)
            defer(0, 15, "q01", qt_mm, 0, 1, 2)
            # qT for the next pair drains across the two jobs before it
            for pr in range(1, PAIRS):
                j = 2 * pr - 2
                defer(j + 1, 4, f"q{pr}0", qt_mm, pr, 0, 0)
                defer(j + 1, 5, f"q{pr}0", qt_mm, pr, 0, 2)
                defer(j + 1, 8, f"q{pr}1", qt_mm, pr, 1, 0)
                defer(j + 1, 9, f"q{pr}1", qt_mm, pr, 1, 2)
            # o-proj for qc0 hides in the last job (ot(0,p3) lands ~slot 5)
            oslots = [(5, 6, 7), (8, 9, 10), (11, 12, 13), (13, 14, 15)]
            for m in range(4):
                a, b, f = oslots[m]
                defer(7, a, f"o{m}", oproj_mm, 0, m, 0)
                defer(7, b, f"o{m}", oproj_mm, 0, m, 2)
                defer(7, f, f"o{m}", oproj_fin, 0, m)

            # ---- 8 attention jobs: qc-major within pair so each pair's qT
            # is reused by consecutive jobs ----
            jobs = [(qc, pr) for pr in range(PAIRS) for qc in range(2)]
            for j, (qc, pr) in enumerate(jobs):
                pA = pacc.tile([128, 512], fp32, tag="pacc")
                pB = pacc.tile([128, 512], fp32, tag="pacc")
                e_tiles = [None] * NKB

                def attnv(kb):
                    e = e_tiles[kb]
                    nc.tensor.matmul(pA[0:65, :], vpt[kb][:, 0:65],
                                     e[:, 0:512],
                                     start=(kb == 0), stop=(kb == NKB - 1))
                    nc.tensor.matmul(pB[0:65, :], vpt[kb][:, 65:130],
                                     e[:, 512:1024],
                                     start=(kb == 0), stop=(kb == NKB - 1))

                for kb in range(NKB):
                    sc_ps = pssc.tile([128, 1024], fp32, tag="scores")
                    nc.tensor.matmul(
                        sc_ps[:, 0:512],
                        ktt[kb // 4][0:64, (kb % 4) * 128:(kb % 4 + 1) * 128],
                        qtt[pr][qc][0:64, :])
                    nc.tensor.matmul(
                        sc_ps[:, 512:1024],
                        ktt[kb // 4][64:128, (kb % 4) * 128:(kb % 4 + 1) * 128],
                        qtt[pr][qc][64:128, :])
                    e = epool.tile([128, 1024], bf16, tag="E")
                    e_tiles[kb] = e
                    nc.scalar.activation(e[:], sc_ps[:], Exp, scale=SCALE)
                    # consume probs two blocks back so PE never waits on exp
                    if kb >= 2:
                        attnv(kb - 2)
                    for fn in deferred[j].get(kb, ()):
                        fn()
                attnv(NKB - 2)
                attnv(NKB - 1)

                # normalize: den_p in pA row 64, den_p+4 in pB row 64.  Copy
                # the live rows to SBUF immediately so the PSUM accumulators
                # free for the next job (pacc is single-buffered), then DMA
                # the den rows to partition 0 (engines cannot shift
                # partitions; the gpsimd broadcast ucode reads partition 0).
                cpA = cpool.tile([65, 512], fp32, tag="cpA")
                cpB = cpool.tile([65, 512], fp32, tag="cpB")
                nc.vector.tensor_copy(cpA[:], pA[0:65, :])
                nc.vector.tensor_copy(cpB[:], pB[0:65, :])
                dq = nc.scalar if j == 7 else nc.sync
                d0 = npool.tile([1, 1024], fp32, tag="den0")
                dq.dma_start(d0[0:1, 0:512], cpA[64:65, :])
                dq.dma_start(d0[0:1, 512:1024], cpB[64:65, :])
                r0 = npool.tile([1, 1024], fp32, tag="rden0")
                nc.vector.reciprocal_approx_fast(out=r0[:], in_=d0[:])
                rbcA = bcpool.tile([64, 512], fp32, tag="rbcA")
                rbcB = bcpool.tile([64, 512], fp32, tag="rbcB")
                nc.gpsimd.partition_broadcast(rbcA[:], r0[0:1, 0:512],
                                              channels=64)
                nc.gpsimd.partition_broadcast(rbcB[:], r0[0:1, 512:1024],
                                              channels=64)
                # assemble both normalized halves into one [128, 512] tile
                # (second half via DMA partition shift) so o-proj contracts
                # over all 128 dims at once
                ot = opool.tile([128, 512], bf16, tag="ot")
                nc.vector.tensor_mul(ot[0:64, :], cpA[0:64, :], rbcA[:])
                obt = obpool.tile([64, 512], bf16, tag="obt")
                nc.vector.tensor_mul(obt[:], cpB[0:64, :], rbcB[:])
                dq.dma_start(ot[64:128, :], obt[:])
                ot_tiles[(qc, pr)] = ot

            # tail: o-proj for qc1 (qc0's was drained into job 7).  The
            # pr=0..2 partials only need ots that are long ready, so they
            # run while job 7's normalize chain produces ot(1,p3); only the
            # final pr=3 matmuls wait on it.
            tb = {}
            for m in range(4):
                tb[m] = chain_b = {}
                if m >= 2:
                    chain_b["ps"] = pacc.tile([128, 512], fp32,
                                              name=f"ypt{m}", tag="pacc")
                oproj_mm(1, m, 0, chain_b)  # pr 0,1
            for m in range(4):
                ps = tb[m]["ps"]
                nc.tensor.matmul(ps[:],
                                 ot_tiles[(1, 2)][:, m * 128:(m + 1) * 128],
                                 wo_sb[:, 2, :], start=False, stop=False)
                nc.tensor.matmul(ps[:],
                                 ot_tiles[(1, 3)][:, m * 128:(m + 1) * 128],
                                 wo_sb[:, 3, :], start=False, stop=True)
                oproj_fin(1, m, tb[m])

    nc.finalize()
    return nc


def _get_nc():
    if "nc" not in _built:
        _built["nc"] = _build_nc()
    return _built["nc"]


def _in_maps(x, Wq, bq, Wk, bk, Wv, bv, Wo, bo):
    import ml_dtypes

    b16 = ml_dtypes.bfloat16
    x = np.ascontiguousarray(np.asarray(x, np.float32))
    Wq = np.asarray(Wq, np.float32)
    bq = np.asarray(bq, np.float32)
    Wk = np.asarray(Wk, np.float32)
    bk = np.asarray(bk, np.float32)
    Wv = np.asarray(Wv, np.float32)
    bv = np.asarray(bv, np.float32)
    Wo = np.asarray(Wo, np.float32)
    bo = np.asarray(bo, np.float32)

    def chunked(a):  # [D, n] row-major -> [128, 4*n] with row d = (c, p)
        n = a.shape[1]
        return np.ascontiguousarray(
            a.reshape(4, 128, n).transpose(1, 0, 2).reshape(128, 4 * n))

    wq_p = chunked(
        Wq.reshape(D, H, DH)[:, PERM, :].reshape(D, D)).astype(b16)
    wo_p = chunked(
        Wo.reshape(H, DH, D)[PERM].reshape(D, D)).astype(b16)
    wk_p = chunked(Wk).astype(b16)
    wv_p = chunked(Wv).astype(b16)
    bq_p = np.ascontiguousarray(
        bq.reshape(H, DH)[PERM].reshape(PAIRS, 128).T)
    bkv_p = np.ascontiguousarray(bk.reshape(128, 1))
    bv_bc = np.ascontiguousarray(np.tile(bv[None, :], (128, 1)))
    bo_bc = np.ascontiguousarray(np.tile(bo[None, :], (128, 1)))

    in_maps = []
    for c in range(NCORES):
        b, sh = divmod(c, 2)
        xroll = np.roll(x[b], -sh * SQ, axis=0)
        # xT [D, S] -> [128, sc, c, 512] chunk-major contiguous
        xprep = np.ascontiguousarray(
            xroll.T.reshape(4, 128, 4, 512).transpose(1, 2, 0, 3)
            .reshape(128, NSC * 4 * 512)).astype(b16)
        in_maps.append({
            "xp": xprep,
            "wq": wq_p, "wk": wk_p, "wv": wv_p, "wo": wo_p,
            "bqp": bq_p, "bkvp": bkv_p, "bvbc": bv_bc, "bobc": bo_bc,
        })
    return in_maps


def kernel(x, Wq, bq, Wk, bk, Wv, bv, Wo, bo):
    from concourse.bass_utils import run_bass_kernel_spmd

    in_maps = _in_maps(x, Wq, bq, Wk, bk, Wv, bv, Wo, bo)
    nc = _get_nc()
    res = run_bass_kernel_spmd(nc, in_maps, list(range(NCORES)))
    out = np.empty((B, S, D), np.float32)
    for c in range(NCORES):
        b, sh = divmod(c, 2)
        out[b, sh * SQ:(sh + 1) * SQ, :] = res.results[c]["y"]
    return out
